# revision 12
# baseline (speedup 1.0000x reference)
"""AttentionPool (pyg-style softmax attention pooling) on 8 Trainium2 cores.

Reference computation:
    s = tanh(x @ W1 + b1) @ W2 + b2            # (N,) node scores
    w = segment_softmax(s, batch)              # per-graph softmax
    out[g] = sum_{i in g} w_i * x[i]           # (B, D)

Design (v4 — column-tiled pooling, PE offload):
  * |s| <= sum|W2| + |b2| <= 8.25 so exp() cannot overflow in fp32 and
    the segment-max subtraction is a mathematical no-op: the segment
    softmax reduces to plain segment sums.
  * batch is sorted -> shard 64 consecutive graphs per core (whole
    graphs on one device); host zero-pads each shard to a common npad
    (pad rows get graph id 64 -> contribute nothing) and concatenates
    the per-core outputs.
  * Both layouts of x ship as fp8 e3m4 (x pre-scaled by 2) packed in
    one 4 KB/partition block per 512 nodes: natural (pool moving
    operand) and transposed (scorer moving operand).  W1/W2 ride in
    bf16 (tiny, and bf16 moving runs at the same 1 col/cycle as fp8).
  * Pooling matmuls out += E_c^T @ x_c have M=64 (64 graphs) — half
    the PE array.  v4 runs them as column-tiled concurrent pairs:
    even tiles at tile_position (0,0) accumulate into PSUM partitions
    0-63, odd tiles at (0,64) into partitions 64-127 (the PE executes
    both simultaneously in disjoint column groups).  The two partial
    accumulators and the DVE-held denominator ship to the host, which
    finishes out = (A+B)/(2*den) during the gather.
  * The denominator leaves the PE entirely: DVE accumulates
    den_acc[p,g] += E_t[p,g] per tile, host reduces over p.
  * Block chain load -> scorer -> score -> pool is software-pipelined
    in pair-batched stages as in v3; per-block DMAs alternate between
    the sync and gpsimd HWDGE queue sets.

Self-contained: hardcodes D=512, H=64, B=512, 8 cores; shard padding
adapts to the runtime batch vector.  loop_M is a timing-only variant
(repeats the body in a hardware loop) used by test.py, never by
kernel().
"""

import numpy as np

D = 512
H = 64
B_GRAPHS = 512
NCORES = 8
G = B_GRAPHS // NCORES

XSCALE = 2.0

_cache = {}


def _build(npad, b2val, loop_M=None):
    import concourse.bacc as bacc
    import concourse.bass as bass
    import concourse.mybir as mybir
    import concourse.tile as tile
    from contextlib import ExitStack

    f32 = mybir.dt.float32
    bf16 = mybir.dt.bfloat16
    f8 = mybir.dt.float8e3
    T = npad // 128
    NB = npad // 512
    AF = mybir.ActivationFunctionType
    ALU = mybir.AluOpType

    nc = bacc.Bacc("TRN2", debug=False)

    # packed per-block payload: [natural (128,2048) | transposed (128,2048)]
    # both fp8 e3m4, both carrying x*XSCALE
    xcd = nc.dram_tensor("xc", [NB, 128, 8 * D], f8, kind="ExternalInput")
    # w1: [p, k*H+h] = W1[k*128+p, h], bf16 (scorer stationary chunks)
    w1d = nc.dram_tensor("w1", [128, 4 * H], bf16, kind="ExternalInput")
    b1d = nc.dram_tensor("b1", [H, 1], f32, kind="ExternalInput")
    w2d = nc.dram_tensor("w2", [H, 1], bf16, kind="ExternalInput")
    btd = nc.dram_tensor("bt", [128, T], f32, kind="ExternalInput")
    giod = nc.dram_tensor("gio", [128, G], f32, kind="ExternalInput")
    # misc col0 = b2 (exp bias, f32)
    miscd = nc.dram_tensor("misc", [128, 1], f32, kind="ExternalInput")
    # outputs: raw pooled sums (two column-group halves) + denominator
    outd = nc.dram_tensor("out", [128, D], f32, kind="ExternalOutput")
    dend = nc.dram_tensor("den", [128, G], f32, kind="ExternalOutput")

    with tile.TileContext(nc) as tc, ExitStack() as ctx:
        constp = ctx.enter_context(tc.tile_pool(name="const", bufs=1))
        xp = ctx.enter_context(tc.tile_pool(name="xin", bufs=16))
        wp = ctx.enter_context(tc.tile_pool(name="work", bufs=6))
        ps2 = ctx.enter_context(
            tc.tile_pool(name="ps2", bufs=3, space=bass.MemorySpace.PSUM)
        )
        accp = ctx.enter_context(
            tc.tile_pool(name="acc", bufs=1, space=bass.MemorySpace.PSUM)
        )
        dap = ctx.enter_context(tc.tile_pool(name="dacc", bufs=1))

        w1_sb = constp.tile([128, 4 * H], bf16)
        b1_sb = constp.tile([H, 1], f32)
        w2_sb = constp.tile([H, 1], bf16)
        bt_sb = constp.tile([128, T], f32)
        gio_sb = constp.tile([128, G], f32)
        misc_sb = constp.tile([128, 1], f32)

        early = {}

        def stage_load(b):
            xc = xp.tile([128, 8 * D], f8, tag="xc")
            # alternate issuing engine to spread transfers over both
            # HWDGE queue sets
            eng = nc.sync if b % 2 == 0 else nc.gpsimd
            eng.dma_start(out=xc[:], in_=xcd.ap()[b])
            live[b] = {"xb": xc[:, 0:4 * D], "xT": xc[:, 4 * D:8 * D]}

        live = {}
        for b0 in range(min(6, NB)):
            stage_load(b0)
            early[b0] = True

        nc.scalar.dma_start(out=w1_sb[:], in_=w1d.ap())
        nc.scalar.dma_start(out=b1_sb[:], in_=b1d.ap())
        nc.scalar.dma_start(out=w2_sb[:], in_=w2d.ap())
        nc.scalar.dma_start(out=bt_sb[:], in_=btd.ap())
        nc.scalar.dma_start(out=gio_sb[:], in_=giod.ap())
        nc.scalar.dma_start(out=misc_sb[:], in_=miscd.ap())

        b2_ap = misc_sb[:, 0:1]

        # persistent accumulators: pooled sums in PSUM (two col-group
        # halves share one 2KB bank), denominator in SBUF via DVE
        out_pair = accp.tile([128, D], f32)
        den_acc = dap.tile([128, G], f32)

        def stage_scorer(b):
            st = live[b]
            hT_ps = ps2.tile([H, D], f32, tag="hT")
            for k in range(4):
                nc.tensor.matmul(
                    hT_ps[:],
                    w1_sb[:, k * H:(k + 1) * H],
                    st["xT"][:, k * 512:(k + 1) * 512],
                    start=(k == 0),
                    stop=(k == 3),
                )
            hT_sb = wp.tile([H, D], bf16, tag="hTs")
            nc.scalar.activation(
                hT_sb[:], hT_ps[:], AF.Tanh, bias=b1_sb[:],
                scale=1.0 / XSCALE,
            )
            st["hT"] = hT_sb

        def stage_score(b):
            st = live[b]
            s_ps = ps2.tile([128, 4], f32, tag="sps")
            for c in range(4):
                nc.tensor.matmul(
                    s_ps[:, c:c + 1],
                    st["hT"][:, c * 128:(c + 1) * 128],
                    w2_sb[:],
                    start=True,
                    stop=True,
                )
            e_sb = wp.tile([128, 4], f32, tag="e")
            nc.scalar.activation(e_sb[:], s_ps[:], AF.Exp, bias=b2_ap)
            st["e"] = e_sb

        def stage_pool(b):
            st = live[b]
            xb, e_sb = st["xb"], st["e"]
            E_sb = wp.tile([128, 4 * G], bf16, tag="E")
            for c in range(4):
                t = b * 4 + c
                nc.vector.tensor_scalar(
                    E_sb[:, c * G:(c + 1) * G],
                    gio_sb[:],
                    bt_sb[:, t:t + 1],
                    e_sb[:, c:c + 1],
                    ALU.is_equal,
                    ALU.mult,
                )
                nc.vector.tensor_tensor(
                    den_acc[:],
                    den_acc[:],
                    E_sb[:, c * G:(c + 1) * G],
                    ALU.add,
                )
                half = c % 2
                first = (b == 0 and c == half)
                last = (b == NB - 1 and c == half + 2)
                nc.tensor.matmul(
                    out_pair[half * G:(half + 1) * G, :],
                    E_sb[:, c * G:(c + 1) * G],
                    xb[:, c * D:(c + 1) * D],
                    start=first,
                    stop=last,
                )
            del live[b]

        def pipeline():
            nc.vector.memset(den_acc[:], 0.0)
            # pair-batched emission: each stage handles two blocks per
            # pipeline step so every cross-engine handoff stalls once
            # per pair instead of once per block
            npair = (NB + 1) // 2

            def pair(fn, p):
                for b in (2 * p, 2 * p + 1):
                    if b < NB:
                        fn(b)

            for i in range(npair + 4):
                if i < npair:
                    for b in (2 * i, 2 * i + 1):
                        if b < NB and b not in early:
                            stage_load(b)
                if 0 <= i - 2 < npair:
                    pair(stage_scorer, i - 2)
                if 0 <= i - 3 < npair:
                    pair(stage_score, i - 3)
                if 0 <= i - 4 < npair:
                    pair(stage_pool, i - 4)

        if loop_M is None:
            pipeline()
        else:
            with tc.For_i(0, loop_M, 1):
                early.clear()
                pipeline()

        out_sb = wp.tile([128, D], f32, tag="osb")
        nc.scalar.copy(out_sb[:], out_pair[:])
        nc.gpsimd.dma_start(out=outd.ap(), in_=out_sb[:])
        nc.sync.dma_start(out=dend.ap(), in_=den_acc[:])

    nc.compile()
    return nc


def _shard_inputs(x, W1, b1, W2, b2, batch):
    import ml_dtypes

    bfp = ml_dtypes.bfloat16
    f8p = ml_dtypes.float8_e3m4
    x = np.ascontiguousarray(np.asarray(x, dtype=np.float32))
    W1 = np.asarray(W1, dtype=np.float32)
    b1 = np.asarray(b1, dtype=np.float32).reshape(H, 1)
    W2 = np.asarray(W2, dtype=np.float32).reshape(H, 1)
    b2val = float(np.asarray(b2).reshape(-1)[0])
    batch = np.asarray(batch).astype(np.int64)

    bounds = np.searchsorted(batch, np.arange(0, B_GRAPHS + 1, G))
    counts = np.diff(bounds)
    npad = int(max(512, -(-int(counts.max()) // 512) * 512))
    T = npad // 128
    NB = npad // 512

    f8max = float(ml_dtypes.finfo(f8p).max)
    w1t = np.ascontiguousarray(
        W1.reshape(4, 128, H).transpose(1, 0, 2).reshape(128, 4 * H)
    ).astype(bfp)
    gio = np.tile(np.arange(G, dtype=np.float32), (128, 1))
    misc = np.full((128, 1), b2val, dtype=np.float32)
    w2b = W2.astype(bfp)

    in_maps = []
    for c in range(NCORES):
        s, e = int(bounds[c]), int(bounds[c + 1])
        xs = np.zeros((npad, D), dtype=np.float32)
        xs[: e - s] = x[s:e]
        xq = np.clip(xs * XSCALE, -f8max, f8max).astype(f8p)
        # natural layout: [b, p, cc*512 + d] = xq[b*512 + cc*128 + p, d]
        xn = xq.reshape(NB, 4, 128, D).transpose(0, 2, 1, 3).reshape(
            NB, 128, 4 * D
        )
        # transposed layout: [b, p, k*512 + n] = xq[b*512 + n, k*128 + p]
        xt = xq.reshape(NB, 512, 4, 128).transpose(0, 3, 2, 1).reshape(
            NB, 128, 4 * D
        )
        xc = np.ascontiguousarray(np.concatenate([xn, xt], axis=2))
        bt = np.full((npad,), float(G), dtype=np.float32)
        bt[: e - s] = (batch[s:e] - c * G).astype(np.float32)
        bt = np.ascontiguousarray(bt.reshape(T, 128).T)
        in_maps.append(
            {
                "xc": xc,
                "w1": w1t,
                "b1": b1,
                "w2": w2b,
                "bt": bt,
                "gio": gio,
                "misc": misc,
            }
        )
    return in_maps, npad, b2val


def run_spmd(x, W1, b1, W2, b2, batch, trace=False, **trace_kwargs):
    from concourse.bass_utils import run_bass_kernel_spmd

    in_maps, npad, b2val = _shard_inputs(x, W1, b1, W2, b2, batch)
    key = (npad, b2val)
    if key not in _cache:
        _cache[key] = _build(npad, b2val)
    nc = _cache[key]
    res = run_bass_kernel_spmd(
        nc, in_maps, list(range(NCORES)), trace=trace, **trace_kwargs
    )
    return res, npad


def kernel(x, W1, b1, W2, b2, batch, B=None, **_unused):
    res, _ = run_spmd(x, W1, b1, W2, b2, batch, trace=False)
    outs = []
    for c in range(NCORES):
        r = res.results[c]
        raw = np.asarray(r["out"], dtype=np.float32)
        den = np.asarray(r["den"], dtype=np.float32).sum(axis=0)
        num = raw[:G] + raw[G:]
        outs.append(num / (XSCALE * den[:, None] + 1e-16))
    return np.concatenate(outs, axis=0).astype(np.float32)
